# revision 3
# baseline (speedup 1.0000x reference)
"""Causal multi-head self-attention (B=32, T=512, C=1024, H=16) on 8 trn2 cores.

Strategy: data-parallel over batch (4 items/core), identical NEFF on all
cores.  All activations are kept in [channel, token] layout on device so
every matmul has its contraction dim on partitions with no transposes:

  QT/KT  = W^T-tiles.T @ xT-tiles           (fp32r, full speed at N>=256)
  S_T    = K_slice.T @ Q_slice  [k, q]      (fp32r; causal => shrink N per kt)
  att    = exp(scale*S_T + pad_bias[k])     (ACT; pad mask is a per-partition bias)
  y/den  = [V | 1].T @ att                  (bf16; ones column gives softmax denom)
  yT     = y * bcast(1/den)                 (PE broadcast matmul + DVE mult)
  outT   = Wp^T-tiles.T @ yT + bp_eff       (bf16)

bq/bk are fused into the PSUM evacuation; bv is folded into
bp_eff = bp + Wp @ bv on the host (valid because softmax rows sum to 1).
"""

import sys

sys.path.insert(0, "/opt/trn_rl_repo")

import ml_dtypes
import numpy as np

import concourse.bass as bass
import concourse.tile as tile
from concourse import bacc, mybir

B, T, C, H = 32, 512, 1024, 16
D = C // H  # 64
N_CORES = 8
BL = B // N_CORES  # batches per core
NEG = -1.0e9

F32 = mybir.dt.float32
F32R = mybir.dt.float32r
BF16 = mybir.dt.bfloat16
BF16_NP = ml_dtypes.bfloat16


def build_nc(c=C, t=T, bl=BL, h=H):
    """Build the per-core Bass program. Same NEFF runs on every core."""
    nct = c // 128   # channel tiles
    ktt = t // 128   # key/token tiles per sequence
    tl = bl * t      # tokens per core
    nch = (c + 511) // 512  # 512-wide output chunks for V projection
    scale = 1.0 / float(np.sqrt(D))

    nc = bacc.Bacc(None, target_bir_lowering=False)

    xT = nc.dram_tensor("xT", [c, tl], F32R, kind="ExternalInput")
    xTb = nc.dram_tensor("xTb", [c, tl], BF16, kind="ExternalInput")
    wq_t = nc.dram_tensor("wq_t", [c, c], F32R, kind="ExternalInput")
    wk_t = nc.dram_tensor("wk_t", [c, c], F32R, kind="ExternalInput")
    wv_t = nc.dram_tensor("wv_t", [c, c], BF16, kind="ExternalInput")
    wp_t = nc.dram_tensor("wp_t", [c, c], BF16, kind="ExternalInput")
    bq_t = nc.dram_tensor("bq_t", [128, nct], F32, kind="ExternalInput")
    bk_t = nc.dram_tensor("bk_t", [128, nct], F32, kind="ExternalInput")
    bpe_t = nc.dram_tensor("bpe_t", [128, nct], F32, kind="ExternalInput")
    pad_t = nc.dram_tensor("pad_t", [128, bl * ktt], F32, kind="ExternalInput")
    cmask = nc.dram_tensor("cmask", [128, 128], F32, kind="ExternalInput")
    outT = nc.dram_tensor("outT", [bl, c, t], F32, kind="ExternalOutput")

    with tile.TileContext(nc) as tc:
        with (
            tc.tile_pool(name="weights", bufs=1) as wpool,
            tc.tile_pool(name="consts", bufs=1) as cpool,
            tc.tile_pool(name="acts", bufs=1) as apool,
            tc.tile_pool(name="att", bufs=4) as attp,
            tc.tile_pool(name="small", bufs=4) as spool,
            tc.tile_pool(name="oevac", bufs=3) as opool,
            tc.tile_pool(name="psum", bufs=8, space=bass.MemorySpace.PSUM) as pp,
        ):
            # ---- load weights / constants once ----
            wq_sb = wpool.tile([128, nct, c], F32R, tag="wq")
            wk_sb = wpool.tile([128, nct, c], F32R, tag="wk")
            wv_sb = wpool.tile([128, nct, c], BF16, tag="wv")
            wp_sb = wpool.tile([128, nct, c], BF16, tag="wp")
            nc.sync.dma_start(wq_sb, wq_t[:].rearrange("(k p) m -> p k m", p=128))
            nc.sync.dma_start(wk_sb, wk_t[:].rearrange("(k p) m -> p k m", p=128))
            nc.sync.dma_start(wv_sb, wv_t[:].rearrange("(k p) m -> p k m", p=128))
            nc.sync.dma_start(wp_sb, wp_t[:].rearrange("(k p) m -> p k m", p=128))

            bq_sb = cpool.tile([128, nct], F32, tag="bq")
            bk_sb = cpool.tile([128, nct], F32, tag="bk")
            bpe_sb = cpool.tile([128, nct], F32, tag="bpe")
            pad_sb = cpool.tile([128, bl * ktt], F32, tag="pad")
            cm_sb = cpool.tile([128, 128], F32, tag="cmask")
            nc.sync.dma_start(bq_sb, bq_t[:])
            nc.sync.dma_start(bk_sb, bk_t[:])
            nc.sync.dma_start(bpe_sb, bpe_t[:])
            nc.sync.dma_start(pad_sb, pad_t[:])
            nc.sync.dma_start(cm_sb, cmask[:])
            ones_sb = cpool.tile([1, 64], F32, tag="ones")
            nc.vector.memset(ones_sb, 1.0)

            for b in range(bl):
                # ---- load this batch's activations ----
                x_sb = apool.tile([128, nct, t], F32R, tag="x")
                xb_sb = apool.tile([128, nct, t], BF16, tag="xb")
                nc.sync.dma_start(
                    x_sb, xT[:, b * t : (b + 1) * t].rearrange("(k p) n -> p k n", p=128)
                )
                nc.sync.dma_start(
                    xb_sb, xTb[:, b * t : (b + 1) * t].rearrange("(k p) n -> p k n", p=128)
                )

                # ---- Q/K projections (fp32r) -> [c, t] layout ----
                qT_sb = apool.tile([128, nct, t], F32R, tag="qT")
                kT_sb = apool.tile([128, nct, t], F32R, tag="kT")
                for dst, w_sb, b_sb in ((qT_sb, wq_sb, bq_sb), (kT_sb, wk_sb, bk_sb)):
                    for m in range(nct):
                        ps = pp.tile([128, t], F32, tag="ps")
                        for k in range(nct):
                            nc.tensor.matmul(
                                ps,
                                w_sb[:, k, m * 128 : (m + 1) * 128],
                                x_sb[:, k, :],
                                start=(k == 0),
                                stop=(k == nct - 1),
                            )
                        nc.vector.tensor_scalar_add(
                            dst[:, m, :], ps, scalar1=b_sb[:, m : m + 1]
                        )

                # ---- V projection (bf16) -> natural [t, c] layout + ones col ----
                v_sb = apool.tile([128, ktt, h, D + 1], BF16, tag="v")
                nc.vector.memset(v_sb[:, :, :, D : D + 1], 1.0)
                for tt in range(ktt):
                    for ch in range(nch):
                        cw = min(512, c - ch * 512)
                        ps = pp.tile([128, cw], F32, tag="ps")
                        for k in range(nct):
                            nc.tensor.matmul(
                                ps,
                                xb_sb[:, k, tt * 128 : (tt + 1) * 128],
                                wv_sb[:, k, ch * 512 : ch * 512 + cw],
                                start=(k == 0),
                                stop=(k == nct - 1),
                            )
                        nc.vector.tensor_copy(
                            v_sb[:, tt, ch * 8 : ch * 8 + cw // D, 0:D],
                            ps.rearrange("p (hh d) -> p hh d", d=D),
                        )

                # ---- attention, head by head ----
                yT_sb = apool.tile([128, nct, t], BF16, tag="yT")
                for hh in range(h):
                    ct, po = hh // 2, (hh % 2) * 64
                    at_tiles = []
                    for i in range(ktt):
                        n = t - 128 * i
                        ps_s = pp.tile([128, n], F32, tag="ps")
                        nc.tensor.matmul(
                            ps_s,
                            kT_sb[po : po + 64, ct, 128 * i : 128 * (i + 1)],
                            qT_sb[po : po + 64, ct, 128 * i : t],
                            start=True,
                            stop=True,
                        )
                        # causal mask only matters on the diagonal 128x128 block
                        nc.vector.tensor_tensor(
                            ps_s[:, 0:128], ps_s[:, 0:128], cm_sb, op=mybir.AluOpType.add
                        )
                        at = attp.tile([128, t], BF16, tag="at")
                        nc.scalar.activation(
                            at[:, 0:n],
                            ps_s,
                            mybir.ActivationFunctionType.Exp,
                            bias=pad_sb[:, b * ktt + i : b * ktt + i + 1],
                            scale=scale,
                        )
                        at_tiles.append(at)
                    ps_av = pp.tile([D + 1, t], F32, tag="ps")
                    for i in range(ktt):
                        n = t - 128 * i
                        nc.tensor.matmul(
                            ps_av[:, 128 * i : t],
                            v_sb[:, i, hh, :],
                            at_tiles[i][:, 0:n],
                            start=(i == 0),
                            stop=(i == ktt - 1),
                        )
                    rec = spool.tile([1, t], F32, tag="rec")
                    nc.vector.reciprocal(rec, ps_av[D : D + 1, :])
                    ps_bc = pp.tile([64, t], F32, tag="ps")
                    nc.tensor.matmul(ps_bc, ones_sb, rec, start=True, stop=True)
                    rb = spool.tile([64, t], F32, tag="rb")
                    nc.any.tensor_copy(rb, ps_bc)
                    nc.vector.tensor_tensor(
                        yT_sb[po : po + 64, ct, :],
                        ps_av[0:D, :],
                        rb,
                        op=mybir.AluOpType.mult,
                    )

                # ---- output projection (bf16) ----
                for m in range(nct):
                    ps = pp.tile([128, t], F32, tag="ps")
                    for k in range(nct):
                        nc.tensor.matmul(
                            ps,
                            wp_sb[:, k, m * 128 : (m + 1) * 128],
                            yT_sb[:, k, :],
                            start=(k == 0),
                            stop=(k == nct - 1),
                        )
                    ot = opool.tile([128, t], F32, tag="ot")
                    nc.vector.tensor_scalar_add(ot, ps, scalar1=bpe_sb[:, m : m + 1])
                    nc.sync.dma_start(outT[b, m * 128 : (m + 1) * 128, :], ot)

    nc.compile()
    return nc


def _prep_core_inputs(x_local, kpm_local, Wq, bq, Wk, bk, Wv, bv, Wp, bp, c=C, t=T, bl=BL):
    """Host-side packing of one core's inputs."""
    nct = c // 128
    ktt = t // 128
    xT = np.ascontiguousarray(x_local.transpose(2, 0, 1).reshape(c, bl * t), dtype=np.float32)
    pad = np.where(kpm_local, np.float32(NEG), np.float32(0.0)).astype(np.float32)  # [bl, t]
    # pad_t[p, b*ktt + i] = pad[b, i*128 + p]
    pad_t = np.ascontiguousarray(
        pad.reshape(bl, ktt, 128).transpose(2, 0, 1).reshape(128, bl * ktt)
    )
    return {
        "xT": xT,
        "xTb": xT.astype(BF16_NP),
        "pad_t": pad_t,
    }


def _prep_shared_inputs(Wq, bq, Wk, bk, Wv, bv, Wp, bp, c=C):
    nct = c // 128
    Wq = np.asarray(Wq, dtype=np.float32)
    Wk = np.asarray(Wk, dtype=np.float32)
    Wv = np.asarray(Wv, dtype=np.float32)
    Wp = np.asarray(Wp, dtype=np.float32)
    bq = np.asarray(bq, dtype=np.float32)
    bk = np.asarray(bk, dtype=np.float32)
    bv = np.asarray(bv, dtype=np.float32)
    bp = np.asarray(bp, dtype=np.float32)
    bp_eff = bp + Wp @ bv
    cm = np.where(
        np.arange(128)[:, None] <= np.arange(128)[None, :], np.float32(0.0), np.float32(NEG)
    ).astype(np.float32)

    def btile(v):
        return np.ascontiguousarray(v.reshape(nct, 128).T)

    return {
        "wq_t": np.ascontiguousarray(Wq.T),
        "wk_t": np.ascontiguousarray(Wk.T),
        "wv_t": np.ascontiguousarray(Wv.T.astype(BF16_NP)),
        "wp_t": np.ascontiguousarray(Wp.T.astype(BF16_NP)),
        "bq_t": btile(bq),
        "bk_t": btile(bk),
        "bpe_t": btile(bp_eff),
        "cmask": cm,
    }


_NC_CACHE = {}


def _get_nc(key=(C, T, BL, H)):
    if key not in _NC_CACHE:
        _NC_CACHE[key] = build_nc(*key)
    return _NC_CACHE[key]


def kernel(x, key_padding_mask, Wq, bq, Wk, bk, Wv, bv, Wp, bp):
    from concourse.bass_utils import run_bass_kernel_spmd

    x = np.asarray(x, dtype=np.float32)
    kpm = np.asarray(key_padding_mask).astype(bool)

    shared = _prep_shared_inputs(Wq, bq, Wk, bk, Wv, bv, Wp, bp)
    in_maps = []
    for cid in range(N_CORES):
        sl = slice(cid * BL, (cid + 1) * BL)
        m = _prep_core_inputs(x[sl], kpm[sl], Wq, bq, Wk, bk, Wv, bv, Wp, bp)
        m.update(shared)
        in_maps.append(m)

    nc = _get_nc()
    res = run_bass_kernel_spmd(nc, in_maps, core_ids=list(range(N_CORES)))

    out = np.empty((B, T, C), dtype=np.float32)
    for cid in range(N_CORES):
        o = res.results[cid]["outT"]  # [BL, C, T]
        out[cid * BL : (cid + 1) * BL] = o.transpose(0, 2, 1)
    return out
